# revision 2
# baseline (speedup 1.0000x reference)
"""Trainium2 Bass kernel for nn_ClusteringLayer (retrieval_knn) — v5.

For each of K=256 clusters, find the nearest of N=100000 points (F=256
features) and return its feature row.

v5 = v1's proven software-pipelined skeleton (per-block PSUM pool tiles,
bufs=2, lag-1 drains; PE never starves) with a drain that is ~2x cheaper
than v1's cast+scan:
  - per 1024-col block, per kc half [128, 1024] fp32 PSUM:
      * one half goes to ScalarE: activation(Exp, bias=-B,
        accum_out=sum) — a log-sum-exp whose log upper/lower-bounds the
        block max, draining+reducing at 1 elem/cyc/lane in ONE pass.
      * the other half goes to DVE: tensor_scalar max-accum (1x) — the
        exact block max.
    Halves alternate per block so both engines carry half the scores.
  - the host brackets exact-max columns tightly and log-sum-exp columns
    with a ln(1024) slack, then rescores candidates exactly in fp64.

Soundness of the exp channel: exp underflow (score < B-87) can zero a
column's accumulator, reporting U=-inf; that only misleads if a
cluster's global best score were < B-87+margin ~ 77. For randn inputs
every cluster has thousands of points scoring > 77 (P[miss] ~ e^-300).
Overflow (score > B+88) makes acc=inf: U=+inf (always rescored), L
clamped to -inf.
"""

import numpy as np

N = 100000
K = 256
F = 256
NCORES = 8
NLOC = N // NCORES            # 12500
BLK = 1024
NFULL = 12
LASTW = 256
NBLK = NFULL + 1              # 13
NPAD = NFULL * BLK + LASTW    # 12544
XSQ_CENTER = float(F)
RESCORE_MARGIN = 14.0
EXP_B = 150.0

SC_KC = [b % 2 for b in range(NBLK)]   # which kc ScalarE exp-drains
COLMAP = [(BLK * b, BLK) for b in range(NFULL)] + [(NFULL * BLK, LASTW)]
NCOLS = len(COLMAP)           # 13
COLTYPE = {kc: ["E" if SC_KC[b] == kc else "M" for b in range(NBLK)]
           for kc in range(2)}

_CACHE = {}


def _build(loop_R=None, _skip=(), lag=1, h_major=False, dma_batch=1,
           resident=False, fused_ps=False):
    import concourse.bass as bass
    import concourse.tile as tile
    from concourse import bacc, mybir

    f32 = mybir.dt.float32
    f16 = mybir.dt.float16
    f8 = mybir.dt.float8e4
    Alu = mybir.AluOpType
    Act = mybir.ActivationFunctionType
    DR = mybir.MatmulPerfMode.DoubleRow

    nc = bacc.Bacc("TRN2", target_bir_lowering=False, debug=False,
                   num_devices=NCORES)

    xq = nc.dram_tensor("xq", [128, NBLK * 2 * BLK], f8,
                        kind="ExternalInput").ap()
    cq = nc.dram_tensor("cq", [128, 2 * K], f8, kind="ExternalInput").ap()
    outs = {}
    for kc in range(2):
        outs[kc] = nc.dram_tensor(f"out_bmax{kc}", [128, NCOLS], f32,
                                  kind="ExternalOutput").ap()

    with tile.TileContext(nc) as tc:
        with (
            tc.tile_pool(name="const", bufs=1) as constp,
            tc.tile_pool(name="xin", bufs=4) as xinp,
            tc.tile_pool(name="fix", bufs=1) as fixp,
            tc.tile_pool(name="psum", bufs=2, space="PSUM") as psp,
        ):
            cqs = constp.tile([128, 2 * K], f8)
            nc.sync.dma_start(cqs[:], cq[:, :])
            cq3 = cqs[:, :].rearrange("p (j m) -> p j m", j=2)
            biasT = constp.tile([128, 1], f32, tag="biasT", name="biasT")
            nc.gpsimd.memset(biasT[:], -EXP_B)

            bm = [fixp.tile([128, NCOLS], f32, tag=f"bm{kc}",
                            name=f"bm{kc}") for kc in range(2)]
            if _skip:
                for kc in range(2):
                    nc.gpsimd.memset(bm[kc][:], 0.0)
            scs = fixp.tile([128, BLK], f16, tag="scs", name="scs")
            scv = fixp.tile([128, BLK], f16, tag="scv", name="scv")

            xq4 = xq[:, :].rearrange("p (b j n) -> p b j n", b=NBLK, j=2)

            def block_body():
                pss = {}
                xts = {}
                xres = None
                if resident:
                    # whole x resident in SBUF (26KB/partition), loaded in
                    # 4 chunks so matmuls start after the first chunk
                    xres = fixp.tile([128, NBLK, 2, BLK], f8, tag="xres",
                                     name="xres")
                    for c0 in range(0, NBLK, 4):
                        nb = min(4, NBLK - c0)
                        nc.sync.dma_start(xres[:, c0:c0 + nb, :, :],
                                          xq4[:, c0:c0 + nb, :, :])
                for step in range(NBLK + lag):
                    if step < NBLK:
                        b = step
                        w = BLK if b < NFULL else LASTW
                        if resident:
                            xall = xres[:, b]
                        elif dma_batch == 1:
                            xall = xinp.tile([128, 2, BLK], f8,
                                             tag="xall", name=f"xall{b}")
                            nc.sync.dma_start(xall[:, :, :w],
                                              xq4[:, b, :, :w])
                        else:
                            if b % dma_batch == 0:
                                nb = min(dma_batch, NBLK - b)
                                xt = xinp.tile([128, dma_batch, 2, BLK],
                                               f8, tag="xall",
                                               name=f"xall{b}")
                                nc.sync.dma_start(
                                    xt[:, :nb, :, :],
                                    xq4[:, b:b + nb, :, :])
                                xts[b] = xt
                            xall = xts[b - b % dma_batch][:, b % dma_batch]
                        pts = {}
                        if fused_ps:
                            ptb = psp.tile([128, 2 * BLK], f32, tag="pk",
                                           name=f"ps{b}")
                            pts[0] = ptb[:, 0:BLK]
                            pts[1] = ptb[:, BLK:2 * BLK]
                        else:
                            for kc in range(2):
                                pt = psp.tile([128, BLK], f32,
                                              tag=f"pk{kc}",
                                              name=f"ps{b}_{kc}")
                                pts[kc] = pt
                        if "mm" not in _skip:
                            if h_major:
                                order = [(kc, h) for h in range(0, w, 512)
                                         for kc in range(2)]
                            else:
                                order = [(kc, h) for kc in range(2)
                                         for h in range(0, w, 512)]
                            for kc, h in order:
                                ks = slice(kc * 128, (kc + 1) * 128)
                                hw = min(512, w - h)
                                nc.tensor.matmul(
                                    pts[kc][:, h:h + hw], cq3[:, :, ks],
                                    xall[:, :, h:h + hw],
                                    start=True, stop=True, perf_mode=DR)
                        pss[b] = (pts, w)
                    bd = step - lag
                    if 0 <= bd < NBLK and "drain" not in _skip:
                        pts, w = pss[bd]
                        kc_s = SC_KC[bd]
                        kc_d = 1 - kc_s
                        if "sc" not in _skip:
                            nc.scalar.activation(
                                scs[:, :w], pts[kc_s][:, :w], Act.Exp,
                                bias=biasT[:, 0:1], scale=1.0,
                                accum_out=bm[kc_s][:, bd:bd + 1])
                        if "dve" not in _skip:
                            nc.vector.tensor_scalar(
                                out=scv[:, :w], in0=pts[kc_d][:, :w],
                                scalar1=1.0, scalar2=-60000.0,
                                op0=Alu.mult, op1=Alu.max,
                                accum_out=bm[kc_d][:, bd:bd + 1])

            if loop_R:
                with tc.For_i(0, loop_R, 1):
                    block_body()
            else:
                block_body()

            for kc in range(2):
                nc.sync.dma_start(outs[kc][:], bm[kc][:])

    nc.compile()
    return nc


def _prep_inputs(x, cluster_centers):
    from concourse import mybir
    f8np = mybir.dt.np(mybir.dt.float8e4)

    x = np.ascontiguousarray(np.asarray(x, dtype=np.float32)).reshape(N, F)
    c = np.asarray(cluster_centers, dtype=np.float32).reshape(K, F)
    xsq = (x.astype(np.float64) ** 2).sum(axis=1).astype(np.float32)

    cq = np.empty((128, 2 * K), np.float32)
    for j in range(2):
        cq[:, j * K:(j + 1) * K] = 2.0 * c[:, j * 128:(j + 1) * 128].T
    cq8 = cq.astype(f8np)

    xT8 = x.T.astype(f8np)
    in_maps = []
    perms = []
    for cidx in range(NCORES):
        lo = cidx * NLOC
        perm = np.argsort(xsq[lo:lo + NLOC], kind="stable")
        perms.append(perm)
        blk8 = np.zeros((2, 128, NPAD), f8np)
        blk8[0, :, :NLOC] = xT8[:128, lo + perm]
        blk8[1, :, :NLOC] = xT8[128:, lo + perm]
        xqb = np.zeros((128, NBLK, 2, BLK), f8np)
        for b in range(NBLK):
            w = BLK if b < NFULL else LASTW
            xqb[:, b, :, :w] = blk8[:, :, b * BLK:b * BLK + w].transpose(
                1, 0, 2)
        xq2 = np.ascontiguousarray(xqb.reshape(128, NBLK * 2 * BLK))
        in_maps.append({"xq": xq2, "cq": cq8})
    _CACHE["perms"] = perms
    return x, c, xsq, in_maps


def _select(xflat, c, xsq, bmax_all):
    """Host combine; per-column device values are either exact raw maxima
    (M) or sums of exp(raw - B) (E). Bracket and rescore in fp64."""
    perms = _CACHE["perms"]
    xsqc64 = xsq.astype(np.float64) - XSQ_CENTER
    c64 = c.astype(np.float64)

    xsq_min = np.zeros((NCORES, NCOLS))
    xsq_max = np.zeros((NCORES, NCOLS))
    for cidx in range(NCORES):
        xs = xsqc64[cidx * NLOC:(cidx + 1) * NLOC][perms[cidx]]
        for j, (lo, wd) in enumerate(COLMAP):
            hi = min(NLOC, lo + wd)
            xsq_min[cidx, j] = xs[lo:hi].min()
            xsq_max[cidx, j] = xs[lo:hi].max()

    U_raw = np.empty_like(bmax_all, dtype=np.float64)
    L_raw = np.empty_like(U_raw)
    with np.errstate(divide="ignore"):
        for kc in range(2):
            for j in range(NCOLS):
                v = bmax_all[:, kc * 128:(kc + 1) * 128, j].astype(
                    np.float64)
                if COLTYPE[kc][j] == "E":
                    u = np.log(v) + EXP_B
                    lw = np.where(np.isfinite(u),
                                  u - np.log(COLMAP[j][1]), -np.inf)
                    U_raw[:, kc * 128:(kc + 1) * 128, j] = u
                    L_raw[:, kc * 128:(kc + 1) * 128, j] = lw
                else:
                    U_raw[:, kc * 128:(kc + 1) * 128, j] = v
                    L_raw[:, kc * 128:(kc + 1) * 128, j] = v

    U = U_raw - xsq_min[:, None, :]
    L = L_raw - xsq_max[:, None, :]
    best = L.max(axis=(0, 2))
    need = U > (best[None, :, None] - RESCORE_MARGIN)

    best_val = np.full(K, -np.inf)
    best_idx = np.zeros(K, np.int64)
    for cidx in range(NCORES):
        for j, (lo, wd) in enumerate(COLMAP):
            kmask = need[cidx, :, j]
            if not kmask.any():
                continue
            hi = min(NLOC, lo + wd)
            if lo >= hi:
                continue
            gidx = cidx * NLOC + perms[cidx][lo:hi]
            xb = xflat[gidx].astype(np.float64)
            ks = np.where(kmask)[0]
            g = 2.0 * (c64[ks] @ xb.T) - xsqc64[gidx][None, :]
            vmax = g.max(axis=1)
            imax = gidx[g.argmax(axis=1)]
            upd = vmax > best_val[ks]
            tie = (vmax == best_val[ks]) & (imax < best_idx[ks])
            sel = upd | tie
            best_val[ks[sel]] = vmax[sel]
            best_idx[ks[sel]] = imax[sel]
    return best_idx


def kernel(x, cluster_centers, _collect_perf=None):
    xflat, c, xsq, in_maps = _prep_inputs(x, cluster_centers)

    if "nc" not in _CACHE:
        _CACHE["nc"] = _build()
    nc = _CACHE["nc"]

    from concourse.bass_utils import run_bass_kernel_spmd
    res = run_bass_kernel_spmd(nc, in_maps, core_ids=list(range(NCORES)))
    if _collect_perf is not None:
        _collect_perf.append(res)

    bmax_all = np.empty((NCORES, K, NCOLS), np.float32)
    for cidx in range(NCORES):
        r = res.results[cidx]
        for kc in range(2):
            bmax_all[cidx, kc * 128:(kc + 1) * 128] = r[f"out_bmax{kc}"]

    final_idx = _select(xflat, c, xsq, bmax_all)
    out = xflat[final_idx].reshape(1, K, F).astype(np.float32)
    return out
